# revision 16
# baseline (speedup 1.0000x reference)
"""DistributionMaxPool Trainium2 kernel.

Math insight: the reference's CxC conv sums the selected 2x2-strided pixel
over ALL input channels and replicates across output channels. Every
per-channel value after that reduction is identical, so the whole
Gaussian-max pipeline runs on channel-summed planes, the device stores
only the [32, 32] pooled grid per (batch, plane), and the host broadcasts
across the 128 channels when assembling the full result.

Per-core pipeline (batch-parallel over 8 cores, 4 batches each):
  1. Each [128, 64*64] plane is cast fp32 -> bf16 during the HBM->SBUF
     DMA (SWDGE cast-load), halving SBUF write traffic. bf16 inputs are
     well inside the 2e-2 tolerance (sum-of-128 rounding noise ~2e-4).
  2. Channel sums via pixels-stationary matmuls with BF16 weights:
     lhsT = single-stride (step 2) view of the plane covering one
     s-parity of 4 consecutive rows (128 pixels), rhs = ones [128, 1],
     N=1. bf16 LDWEIGHTS is ~12x faster than the fp32 4-byte weight
     path measured on this part. Each matmul writes one PSUM column;
     2x16 matmuls per plane fill two [128, 16] PSUM tiles with layout:
       partition p = 64u + 32r + j, col q   ->  pixel (i = 2q + u, j)
     for output pixel (i, j), 2x2 offset (r, s).
  3. PSUM tiles from all 4 batches are evacuated (ACT engine, which
     reads PSUM) into fused [128, 128] sum tiles, col = 64s + 16b + q.
     The Gaussian-max math then runs ONCE per rep, fused across all 4
     batches: stage A (s-parity) on [128, 64] halves; base-align copies;
     stage B (r-parity) as a single [32, 128] call. This cuts the DVE
     instruction count ~4x, which dominates at these tiny tile sizes.
  4. Output: per (batch, plane), interleave the two u-halves into a
     [32, 32] grid (stride-2 single-stride dests), vector-transpose, and
     store the 4 KB grid via the scalar HWDGE queue.
"""

import sys

if "/opt/trn_rl_repo" not in sys.path:
    sys.path.insert(0, "/opt/trn_rl_repo")

import numpy as np

B_FULL = 32
N_CORES = 8
B = B_FULL // N_CORES  # 4 batches per core
C = 128
H = W = 64
HO = WO = 32
NPIX = HO * WO  # 1024

EPS = 1e-8
INV_SQRT2 = float(1.0 / np.sqrt(2.0))
INV_SQRT_2PI = float(1.0 / np.sqrt(2.0 * np.pi))
RSQRT_MAGIC = 0x5F3759DF

CAST_MODE = "tri"  # "swdge": cast in DMA; "engine": fp32 load + DVE/ACT cast

_CACHE = {}


def _gauss_max(nc, pool, m1, v1, m2, v2, out_mean, out_var, P, F):
    """mean/var of max of two Gaussians, elementwise on [P, F] views.

    d = m1-m2, p = m1+m2, s = v1+v2+eps, rs = 1/sqrt(s), alpha = s*rs,
    beta = d*rs, e = erf(beta/sqrt2), g = exp(-beta^2/2):
      mean = p/2 + e*d/2 + c2*alpha*g
      var  = s/2 + (p^2+d^2)/4 + eps + e*(d*p + (v1-v2))/2 + c2*p*alpha*g
             - mean^2
    ACT ops: erf, tanh, square only (single activation table).
    """
    import concourse.mybir as mybir

    f32 = mybir.dt.float32
    i32 = mybir.dt.int32
    Act = mybir.ActivationFunctionType
    mult = mybir.AluOpType.mult
    add = mybir.AluOpType.add
    shr = mybir.AluOpType.arith_shift_right

    def t(name, dtype=f32):
        return pool.tile([P, F], dtype, name=name, tag=f"{name}_{P}x{F}")

    s_ = t("gs")
    nc.vector.scalar_tensor_tensor(s_[:], v1, EPS, v2, add, add)
    # rs = rsqrt(s): bit-trick seed + 3 Newton iterations (fp32-exact)
    sh = t("gsh", i32)
    nc.vector.tensor_scalar(sh[:], s_[:].bitcast(i32), 1, None, shr)
    yi = t("gy", i32)
    nc.vector.tensor_scalar(yi[:], sh[:], -1, RSQRT_MAGIC, mult, add)
    yf = yi[:].bitcast(f32)
    nt1 = t("gnt1")
    nt2 = t("gnt2")
    for _ in range(3):
        nc.vector.tensor_mul(nt1[:], yf, yf)
        nc.vector.scalar_tensor_tensor(nt2[:], nt1[:], -0.5, s_[:], mult, mult)
        nc.vector.scalar_tensor_tensor(yf, nt2[:], 1.5, yf, add, mult)
    alpha = t("galpha")
    nc.vector.tensor_mul(alpha[:], s_[:], yf)
    d = t("gd")
    nc.vector.tensor_sub(d[:], m1, m2)
    beta = t("gbeta")
    nc.vector.tensor_mul(beta[:], d[:], yf)
    e = t("ge")
    nc.scalar.activation(e[:], beta[:], Act.Erf, scale=INV_SQRT2)
    b2 = t("gb2")
    nc.scalar.square(b2[:], beta[:])
    # g = exp(-b2/2) = (1-T)/(1+T), T = tanh(b2/4)
    T = t("gT")
    nc.scalar.activation(T[:], b2[:], Act.Tanh, scale=0.25)
    num = t("gnum")
    nc.vector.tensor_scalar(num[:], T[:], -1.0, 1.0, mult, add)
    den = t("gden")
    nc.vector.tensor_scalar(den[:], T[:], 1.0, None, add)
    dr = t("gdr")
    nc.vector.reciprocal(dr[:], den[:])
    g = t("gg")
    nc.vector.tensor_mul(g[:], num[:], dr[:])

    p_ = t("gp")
    nc.vector.tensor_add(p_[:], m1, m2)
    ag = t("gag")
    nc.vector.tensor_mul(ag[:], alpha[:], g[:])
    # mean = 0.5*p + 0.5*e*d + c2*ag
    u_ = t("gu")
    nc.vector.scalar_tensor_tensor(u_[:], e[:], 0.5, d[:], mult, mult)
    w_ = t("gw")
    nc.vector.scalar_tensor_tensor(w_[:], p_[:], 0.5, u_[:], mult, add)
    nc.vector.scalar_tensor_tensor(out_mean, ag[:], INV_SQRT_2PI, w_[:], mult, add)
    # var
    dv = t("gdv")
    nc.vector.tensor_sub(dv[:], v1, v2)
    dp = t("gdp")
    nc.vector.tensor_mul(dp[:], d[:], p_[:])
    z = t("gz")
    nc.vector.tensor_add(z[:], dp[:], dv[:])
    ez = t("gez")
    nc.vector.scalar_tensor_tensor(ez[:], e[:], 0.5, z[:], mult, mult)
    d2 = t("gd2")
    nc.scalar.square(d2[:], d[:])
    p2 = t("gp2")
    nc.scalar.square(p2[:], p_[:])
    pd = t("gpd")
    nc.vector.tensor_add(pd[:], p2[:], d2[:])
    qd = t("gqd")
    nc.vector.tensor_scalar(qd[:], pd[:], 0.25, EPS, mult, add)
    acc = t("gacc")
    nc.vector.scalar_tensor_tensor(acc[:], s_[:], 0.5, qd[:], mult, add)
    v3 = t("gv3")
    nc.vector.tensor_add(v3[:], ez[:], acc[:])
    pag = t("gpag")
    nc.vector.tensor_mul(pag[:], p_[:], ag[:])
    v4 = t("gv4")
    nc.vector.scalar_tensor_tensor(v4[:], pag[:], INV_SQRT_2PI, v3[:], mult, add)
    v5 = t("gv5")
    nc.scalar.square(v5[:], out_mean)
    nc.vector.scalar_tensor_tensor(out_var, v5[:], -1.0, v4[:], mult, add)


def _kernel_body(nc, tc, x, y, onesb, xin, xf32p, sums, math_pool, outp, psp):
    import concourse.mybir as mybir

    f32 = mybir.dt.float32
    bf16 = mybir.dt.bfloat16

    # Fused channel-sum tiles for all 4 batches: col = 64s + 16b + q
    sm_all = sums.tile([128, 128], f32, name="sm", tag="sm")
    sv_all = sums.tile([128, 128], f32, name="sv", tag="sv")

    for b in range(B):
        for pl in (0, 1):
            xf = x[b, pl].rearrange("c h w -> c (h w)")
            # One full-plane 2 MB cast-load: biggest descriptors, best DMA
            # efficiency; input bandwidth is the binding constraint, and
            # pipeline fill amortizes across the batch loop.
            xb = xin.tile([C, 4096], bf16, name="xb", tag="xb")
            pidx = 2 * b + pl
            # Input rings: a single DMA queue caps well below HBM rate, so
            # spread the 8 plane loads over SWDGE (casts in the DMA) plus
            # both HWDGE queues (fp32, cast on DVE/ACT which have slack).
            if CAST_MODE == "swdge":
                plan = "sw"
            elif CAST_MODE == "tri":
                plan = {
                    0: "sw", 2: "sw", 4: "sw", 6: "sw",
                    1: ("sync", "dve"), 5: ("sync", "dve"),
                    3: ("scalar", "dve"), 7: ("scalar", "act"),
                }[pidx]
            else:  # hwdge2q
                plan = ("sync" if pidx % 2 == 0 else "scalar",
                        "dve" if pidx % 3 != 1 else "act")
            if plan == "sw":
                nc.gpsimd.dma_start(xb[:], xf)
            else:
                ld, ce = plan
                xr = xf32p.tile([C, 4096], f32, name="xr", tag="xr")
                (nc.sync if ld == "sync" else nc.scalar).dma_start(xr[:], xf)
                if ce == "dve":
                    nc.vector.tensor_copy(xb[:], xr[:])
                else:
                    nc.scalar.copy(xb[:], xr[:])
            x3 = xb[:].rearrange("c (q m s) -> c q m s", q=16, m=128, s=2)
            dst = sm_all if pl == 0 else sv_all
            pss = [
                psp.tile([128, 16], f32, name=f"ps{s}", tag="ps")
                for s in range(2)
            ]
            for s in range(2):
                for q in range(16):
                    nc.tensor.matmul(
                        pss[s][:, q : q + 1],
                        x3[:, q, :, s],
                        onesb[:, 0:1],
                        start=True,
                        stop=True,
                    )
            # PSUM -> fused sums (ACT engine reads PSUM, keeps DVE free)
            for s in range(2):
                c0 = 64 * s + 16 * b
                nc.scalar.copy(dst[:, c0 : c0 + 16], pss[s][:])

    # Stage A: s=0 vs s=1, fused over all 4 batches: [128, 64] halves.
    hm = math_pool.tile([128, 64], f32, name="hm", tag="hm")
    hv = math_pool.tile([128, 64], f32, name="hv", tag="hv")
    _gauss_max(
        nc, math_pool,
        sm_all[:, 0:64], sv_all[:, 0:64], sm_all[:, 64:128], sv_all[:, 64:128],
        hm[:], hv[:], 128, 64,
    )
    # Base-align the r=0 / r=1 operands of both u halves into [32, 128]
    # tiles (2-src ops need equal input base partitions), col = 64u+16b+q.
    m1c = math_pool.tile([32, 128], f32, name="m1c", tag="m1c")
    v1c = math_pool.tile([32, 128], f32, name="v1c", tag="v1c")
    m2c = math_pool.tile([32, 128], f32, name="m2c", tag="m2c")
    v2c = math_pool.tile([32, 128], f32, name="v2c", tag="v2c")
    for u in range(2):
        base = 64 * u
        cols = slice(64 * u, 64 * u + 64)
        nc.vector.tensor_copy(m1c[:, cols], hm[base : base + 32, :])
        nc.vector.tensor_copy(v1c[:, cols], hv[base : base + 32, :])
        nc.vector.tensor_copy(m2c[:, cols], hm[base + 32 : base + 64, :])
        nc.vector.tensor_copy(v2c[:, cols], hv[base + 32 : base + 64, :])
    # Stage B: r=0 vs r=1, ONE fused [32, 128] call.
    meant = math_pool.tile([32, 128], f32, name="meant", tag="meant")
    vart = math_pool.tile([32, 128], f32, name="vart", tag="vart")
    _gauss_max(
        nc, math_pool,
        m1c[:], v1c[:], m2c[:], v2c[:],
        meant[:], vart[:], 32, 128,
    )

    # Output per (batch, plane): interleave u-halves to col = i = 2q+u
    # (stride-2 single-stride dests), transpose to [i, j], store 4 KB.
    for b in range(B):
        for pl, src in ((0, meant), (1, vart)):
            tmp = outp.tile([32, 32], f32, name="tp", tag="tp")
            tmp3 = tmp[:].rearrange("p (q u) -> p q u", u=2)
            nc.vector.tensor_copy(tmp3[:, :, 0], src[:, 16 * b : 16 * b + 16])
            nc.vector.tensor_copy(
                tmp3[:, :, 1], src[:, 64 + 16 * b : 64 + 16 * b + 16]
            )
            mt = outp.tile([32, 32], f32, name="mt", tag="mt")
            nc.vector.transpose(mt[:], tmp[:])
            nc.scalar.dma_start(y[b, pl], mt[:])


def _build(reps=1, timing=False):
    import concourse.bacc as bacc
    import concourse.mybir as mybir
    import concourse.tile as tile

    f32 = mybir.dt.float32
    bf16 = mybir.dt.bfloat16
    nc = bacc.Bacc("TRN2", target_bir_lowering=False, debug=False, num_devices=N_CORES)

    x = nc.declare_dram_parameter("x", [B, 2, C, H, W], f32, isOutput=False)
    if timing:
        # Device work identical, but keep y internal and read back only a
        # few bytes so none of the stores are dead.
        y = nc.dram_tensor("y_int", [B, 2, HO, WO], f32)
        y_small = nc.declare_dram_parameter("ysum", [1, 4], f32, isOutput=True)
    else:
        y = nc.declare_dram_parameter("y", [B, 2, HO, WO], f32, isOutput=True)

    with tile.TileContext(nc) as tc:
        with (
            tc.tile_pool(name="xin", bufs=4) as xin,
            tc.tile_pool(name="xf32", bufs=4) as xf32p,
            tc.tile_pool(name="const", bufs=1) as const,
            tc.tile_pool(name="sums", bufs=2) as sums,
            tc.tile_pool(name="math", bufs=2) as math_pool,
            tc.tile_pool(name="out", bufs=4) as outp,
            tc.tile_pool(name="ps", bufs=4, space="PSUM") as psp,
        ):
            onesb = const.tile([128, 1], bf16)
            nc.gpsimd.memset(onesb[:], 1.0)

            for _rep in range(reps):
                _kernel_body(
                    nc, tc, x, y, onesb, xin, xf32p, sums, math_pool, outp, psp
                )

            if timing:
                rb = outp.tile([1, 4], f32, name="rb", tag="rb")
                nc.sync.dma_start(rb[:], y[0, 0][0:1, 0:4])
                nc.sync.dma_start(y_small[:], rb[:])

    nc.compile()
    return nc


def _get_nc():
    if "nc" not in _CACHE:
        _CACHE["nc"] = _build()
    return _CACHE["nc"]


def kernel(x: np.ndarray) -> np.ndarray:
    from concourse.bass_utils import run_bass_kernel_spmd

    assert x.shape == (B_FULL, 2, C, H, W), x.shape
    x = np.ascontiguousarray(x, dtype=np.float32)
    nc = _get_nc()
    in_maps = [{"x": x[i * B : (i + 1) * B]} for i in range(N_CORES)]
    res = run_bass_kernel_spmd(nc, in_maps, list(range(N_CORES)))
    small = np.concatenate(
        [res.results[i]["y"] for i in range(N_CORES)], axis=0
    )  # [B_FULL, 2, HO, WO]
    out = np.empty((B_FULL, 2, C, HO, WO), np.float32)
    out[:] = small[:, :, None, :, :]
    return out
